# revision 32
# baseline (speedup 1.0000x reference)
"""DeepseekV3 MoE layer on 8 Trainium2 NeuronCores (Bass/Tile).

Sharding (v2):
  - Router: data-parallel (each core routes its own T/8=512 tokens, fp32,
    selection on exact logits), AllGather of per-token (sel-mask, weight).
  - Every core computes the FULL [E, T] routing table (rank via DVE prefix
    scan over the sel mask; rank <= C survives, matching the reference's
    stable-sort capacity drop).  From it each core derives:
      * compacted token lists for its own 4 experts (selected via a
        per-core one-hot `esel` matmul + gpsimd sparse_gather)
      * slot ids + weights for its own 512 tokens' top-4 picks (table
        transposed to [T, 128] in DRAM, own columns pulled via dma_gather
        with a per-core `tidx` index list)
  - Routed experts: expert-parallel, 4 experts/core, capacity-exact
    CP=160 slots, bf16 GEMMs (fp32 PSUM accumulate).  Slot outputs
    (unweighted) are written to y_own [640, D] bf16.
  - Combine: AllGather(y_own) -> y_all [5120, D]; each core gathers its
    own tokens' 4 slot rows and folds the routing weights in via
    diagonal-matmul PSUM accumulation, adds the locally computed shared
    expert MLP, writes its output slice.

kernel(**inputs) takes the full unsharded inputs and returns the full
[B, S, D] output.  Self-contained: hardcodes all shapes.
"""

import os
import sys

for _p in ("/opt/trn_rl_repo", "/opt/pypackages"):
    if _p not in sys.path:
        sys.path.insert(0, _p)

import numpy as np

# ---------------------------------------------------------------- constants
B, S, D = 2, 2048, 2048
T = B * S                  # 4096 tokens
I = 1024                   # routed expert intermediate
E = 32                     # routed experts
K = 4                      # experts per token
NG = 8                     # groups
GS = E // NG               # experts per group = 4
TKG = 3                    # top-k groups
ISH = 2048                 # shared expert intermediate
SCALE = 2.5
C = 160                    # capacity = ceil(1.25 * T / E)
CP = 160                   # capacity-exact slots per expert
NCORES = 8
EL = E // NCORES           # local experts per core = 4
TL = T // NCORES           # local tokens per core = 512
NSLOT = EL * C             # slots per core = 640
EC = E * C                 # total slots = 5120

DC = D // 128              # 16
IC = I // 128              # 8
MC = ISH // 128            # 16
TT = TL // 128             # 4 own-token tiles
NT = T // 128              # 32 all-token tiles
IW = CP // 16              # real idx cols per expert = 10
IWT = 16                   # padded idx cols per expert (256 slots, -1 fill)

# "f32" | "bf16" : dtype of the heavy GEMMs (router stays f32)
GEMM_MODE = os.environ.get("BASS_MOE_GEMM_MODE", "bf16")


# ---------------------------------------------------------------- builder
def _build(gemm_mode: str):
    import concourse.bass as bass
    import concourse.bacc as bacc
    import concourse.mybir as mybir
    import concourse.tile as tile
    from concourse import masks
    from contextlib import ExitStack

    dt = mybir.dt
    Alu = mybir.AluOpType
    Act = mybir.ActivationFunctionType

    f32 = dt.float32
    wdt = dt.bfloat16 if gemm_mode == "bf16" else f32

    nc = bacc.Bacc(None, num_devices=NCORES, num_swdge_queues=1)
    groups = [list(range(NCORES))]

    # ---------------- I/O ----------------
    x_full = nc.dram_tensor("x_full", [T, D], f32, kind="ExternalInput")
    x_bf16 = nc.dram_tensor("x_bf16", [T, D], wdt, kind="ExternalInput")
    rwT = nc.dram_tensor("rwT", [D, E], f32, kind="ExternalInput")
    ebias = nc.dram_tensor("ebias", [1, E], f32, kind="ExternalInput")
    esel = nc.dram_tensor("esel", [E, EL], f32, kind="ExternalInput")
    tidx = nc.dram_tensor("tidx", [128, TL // 16], dt.int16,
                          kind="ExternalInput")
    wgu = nc.dram_tensor("wgu", [EL, IC, 128, 2 * DC * 128], wdt,
                         kind="ExternalInput")
    wd = nc.dram_tensor("wd", [EL, I, D], wdt, kind="ExternalInput")
    sgu = nc.dram_tensor("sgu", [MC, 128, 2 * DC * 128], wdt,
                         kind="ExternalInput")
    sdT = nc.dram_tensor("sdT", [ISH, D], wdt, kind="ExternalInput")
    out = nc.dram_tensor("out", [TL, D], f32, kind="ExternalOutput")
    pout = nc.dram_tensor("pout", [T, D], f32, kind="ExternalOutput")

    # ---------------- internal DRAM ----------------
    at_dram = nc.dram_tensor("at_dram", [EL, T], f32)
    aw_dram = nc.dram_tensor("aw_dram", [EL, T], f32)
    wc_dram = nc.dram_tensor("wc_dram", [EL, 2 * 128], f32)
    nf_dram = nc.dram_tensor("nf_dram", [1, EL], f32)
    idx_dram = nc.dram_tensor("idx_dram", [16, EL * IWT], dt.int16)

    with tile.TileContext(nc) as tc, ExitStack() as ctx:
        consts = ctx.enter_context(tc.tile_pool(name="consts", bufs=1))
        work = ctx.enter_context(tc.tile_pool(name="work", bufs=2))
        persist = ctx.enter_context(tc.tile_pool(name="persist", bufs=1))
        psum_t = ctx.enter_context(
            tc.tile_pool(name="psum_t", bufs=2, space="PSUM"))

        # ---------------- constants ----------------
        ident = consts.tile([128, 128], f32)
        masks.make_identity(nc, ident[:])
        if wdt != f32:
            ident_w = consts.tile([128, 128], wdt)
            nc.vector.tensor_copy(ident_w[:], ident[:])
        else:
            ident_w = ident

        ebias_b = consts.tile([128, E], f32)
        nc.sync.dma_start(ebias_b[:], ebias[0:1, :].broadcast_to([128, E]))

        negbuf = consts.tile([128, E], f32)
        nc.gpsimd.memset(negbuf[:], -1e30)

        # iota over wrapped [16, IW] slots: value = p + 16*f
        iota_sl_i = consts.tile([16, IW], dt.int32)
        nc.gpsimd.iota(iota_sl_i[:], pattern=[[16, IW]], base=0,
                       channel_multiplier=1)
        iota_sl = consts.tile([16, IW], f32)
        nc.vector.tensor_copy(iota_sl[:], iota_sl_i[:])

        esel_sb = consts.tile([E, EL], f32)
        nc.sync.dma_start(esel_sb[:], esel[:])
        tidx_sb = consts.tile([128, TL // 16], dt.int16)
        nc.sync.dma_start(tidx_sb[:], tidx[:])

        # expert-dispatch persists
        idx16 = persist.tile([128, EL * IWT], dt.int16)
        w_col = persist.tile([128, 2 * EL], f32)

        # zero-fill the [T, D] partial output early (overlaps compute);
        # the host sums the 8 partial shards (partial-sum unshard).
        zt = consts.tile([128, 512], f32)
        nc.gpsimd.memset(zt[:], 0.0)
        for zr in range(NT):
            for zc in range(D // 512):
                nc.sync.dma_start(
                    pout[zr * 128:(zr + 1) * 128,
                         zc * 512:(zc + 1) * 512], zt[:])

        # ---------------- P1: own tokens -> xTw via transpose-gather ------
        xTw = persist.tile([128, DC, TL], wdt)
        nc.gpsimd.dma_gather(
            xTw[:], x_bf16[:], tidx_sb[:], TL, TL, D,
            transpose=True, queue_num=0)

        # ---------------- P2: replicated router on ALL tokens (fp32) ------
        # Every core routes all T tokens from the fp32 x_full -> identical
        # selw_sb on every core; no routing collective needed.
        rwT_sb = consts.tile([128, DC, E], f32)
        nc.sync.dma_start(
            rwT_sb[:], rwT[:].rearrange("(c p) e -> p c e", p=128))

        selw_sb = persist.tile([128, NT, 2 * E], f32)

        for tt in range(NT):
            xTt = work.tile([128, DC, 128], f32, tag="xTt")
            for dc2 in range(DC // 2):
                xtile = work.tile([128, 256], f32, tag="xtile")
                nc.sync.dma_start(
                    xtile[:],
                    x_full[tt * 128:(tt + 1) * 128,
                           dc2 * 256:(dc2 + 1) * 256])
                for h in range(2):
                    dc = dc2 * 2 + h
                    pt = psum_t.tile([128, 128], f32, tag="pt")
                    nc.tensor.transpose(
                        pt[:], xtile[:, h * 128:(h + 1) * 128], ident[:])
                    nc.vector.tensor_copy(xTt[:, dc, :], pt[:])
            ps = psum_t.tile([128, E], f32, tag="pt")
            for dc in range(DC):
                nc.tensor.matmul(
                    ps[:], xTt[:, dc, :], rwT_sb[:, dc, :],
                    start=(dc == 0), stop=(dc == DC - 1))
            L = work.tile([128, E], f32, tag="rL")
            nc.vector.tensor_copy(L[:], ps[:])
            Ssig = work.tile([128, E], f32, tag="rS")
            nc.scalar.activation(Ssig[:], ps[:], Act.Sigmoid)
            Sb = work.tile([128, E], f32, tag="rSb")
            nc.vector.tensor_tensor(Sb[:], Ssig[:], ebias_b[:], op=Alu.add)

            # group score = top-2 sum per group = max over pair sums
            Sv = Sb[:].rearrange("p (g i) -> p g i", i=GS)
            gs = work.tile([128, NG], f32, tag="rGS")
            tmp = work.tile([128, NG], f32, tag="rtmp")
            nc.vector.tensor_tensor(gs[:], Sv[:, :, 0], Sv[:, :, 1], op=Alu.add)
            for (a, b) in [(0, 2), (0, 3), (1, 2), (1, 3), (2, 3)]:
                nc.vector.tensor_tensor(
                    tmp[:], Sv[:, :, a], Sv[:, :, b], op=Alu.add)
                nc.vector.tensor_tensor(gs[:], gs[:], tmp[:], op=Alu.max)

            m8g = work.tile([128, 8], f32, tag="rm8g")
            nc.vector.max(m8g[:], gs[:])
            gmask = work.tile([128, NG], f32, tag="rgm")
            nc.vector.tensor_scalar(
                gmask[:], gs[:], m8g[:, TKG - 1:TKG], None, op0=Alu.is_ge)

            emask = work.tile([128, E], f32, tag="rem")
            emv = emask[:].rearrange("p (g i) -> p g i", i=GS)
            for r in range(GS):
                nc.vector.tensor_copy(emv[:, :, r], gmask[:])

            # top-4 experts among unmasked, compared on exact logits
            emask8 = work.tile([128, E], dt.uint8, tag="rem8")
            nc.vector.tensor_copy(emask8[:], emask[:])
            ml = work.tile([128, E], f32, tag="rml")
            nc.vector.tensor_copy(ml[:], negbuf[:])
            nc.vector.copy_predicated(ml[:], emask8[:], L[:])
            m8e = work.tile([128, 8], f32, tag="rm8e")
            nc.vector.max(m8e[:], ml[:])
            sel = work.tile([128, E], f32, tag="rsel")
            nc.vector.tensor_scalar(
                sel[:], ml[:], m8e[:, K - 1:K], None, op0=Alu.is_ge)

            wm = work.tile([128, E], f32, tag="rwm")
            nc.vector.tensor_tensor(wm[:], Ssig[:], sel[:], op=Alu.mult)
            den = work.tile([128, 1], f32, tag="rden")
            nc.vector.tensor_reduce(
                den[:], wm[:], axis=mybir.AxisListType.X, op=Alu.add)
            nc.vector.tensor_scalar(den[:], den[:], 1e-20, None, op0=Alu.add)
            winv = work.tile([128, 1], f32, tag="rwinv")
            nc.vector.reciprocal(winv[:], den[:])

            nc.vector.tensor_copy(selw_sb[:, tt, 0:E], sel[:])
            nc.vector.tensor_scalar(
                selw_sb[:, tt, E:2 * E], wm[:], winv[:, 0:1], SCALE,
                op0=Alu.mult, op1=Alu.mult)

        # ---------------- P4..P6: heavy GEMM phases ----------------
        with tc.tile_pool(name="psum_g", bufs=2, space="PSUM") as psum_g, \
                tc.tile_pool(name="psum_u", bufs=2, space="PSUM") as psum_u, \
                tc.tile_pool(name="psum_y", bufs=2, space="PSUM") as psum_y, \
                tc.tile_pool(name="wstream", bufs=2) as wstream:

            # ---- P4: shared expert (gate/up then down) ----
            with tc.tile_pool(name="hst", bufs=1) as hst:
                HsT = hst.tile([128, MC, TL], wdt)
                for mc in range(MC):
                    sgu_t = wstream.tile([128, DC, 256], wdt, tag="wst")
                    nc.sync.dma_start(
                        sgu_t[:],
                        sgu[mc].rearrange("p (c j) -> p c j", j=256))
                    pg = psum_g.tile([128, TL], f32, tag="pg")
                    pu = psum_u.tile([128, TL], f32, tag="pu")
                    for dc in range(DC):
                        nc.tensor.matmul(
                            pg[:], sgu_t[:, dc, 0:128], xTw[:, dc, :],
                            start=(dc == 0), stop=(dc == DC - 1))
                    for dc in range(DC):
                        nc.tensor.matmul(
                            pu[:], sgu_t[:, dc, 128:256], xTw[:, dc, :],
                            start=(dc == 0), stop=(dc == DC - 1))
                    sig = work.tile([128, TL], f32, tag="ssig")
                    nc.scalar.activation(sig[:], pg[:], Act.Sigmoid)
                    sil = work.tile([128, TL], wdt, tag="ssil")
                    nc.vector.tensor_tensor(sil[:], sig[:], pg[:], op=Alu.mult)
                    nc.vector.tensor_tensor(
                        HsT[:, mc, :], sil[:], pu[:], op=Alu.mult)

                # shared down-proj -> shr_out
                with tc.tile_pool(name="sdpool", bufs=1) as sdpool:
                    for dc4 in range(D // 512):
                        sd_t = sdpool.tile([128, MC, 512], wdt, tag="wsd")
                        nc.sync.dma_start(
                            sd_t[:],
                            sdT[:].rearrange("(c p) d -> p c d", p=128)
                            [:, :, dc4 * 512:(dc4 + 1) * 512])
                        for tb in range(TT):
                            po = psum_y.tile([128, 512], f32, tag="py")
                            for mc in range(MC):
                                nc.tensor.matmul(
                                    po[:],
                                    HsT[:, mc, tb * 128:(tb + 1) * 128],
                                    sd_t[:, mc, :],
                                    start=(mc == 0), stop=(mc == MC - 1))
                            ot = work.tile([128, 512], f32, tag="ot")
                            nc.vector.tensor_copy(ot[:], po[:])
                            nc.sync.dma_start(
                                out[tb * 128:(tb + 1) * 128,
                                    dc4 * 512:(dc4 + 1) * 512], ot[:])

            # ---- P5: routing tables (all 32 experts) ----
            with tc.tile_pool(name="route", bufs=1) as route:
                selTs = route.tile([E, T], f32)   # sel mask, [e, t]
                selTw = route.tile([E, T], f32)   # weights,  [e, t]
                for tt in range(NT):
                    pts = psum_t.tile([E, 128], f32, tag="pt")
                    nc.tensor.transpose(
                        pts[:], selw_sb[:, tt, 0:E], ident[:])
                    nc.vector.tensor_copy(
                        selTs[:, tt * 128:(tt + 1) * 128], pts[:])
                    ptw = psum_t.tile([E, 128], f32, tag="pt")
                    nc.tensor.transpose(
                        ptw[:], selw_sb[:, tt, E:2 * E], ident[:])
                    nc.vector.tensor_copy(
                        selTw[:, tt * 128:(tt + 1) * 128], ptw[:])

                rank = route.tile([E, T], f32)
                nc.vector.tensor_tensor_scan(
                    rank[:], selTs[:], selTs[:], 0.0,
                    op0=Alu.add, op1=Alu.bypass)

                valid = route.tile([E, T], f32)
                nc.vector.tensor_scalar(
                    valid[:], rank[:], float(C), None, op0=Alu.is_le)
                nc.vector.tensor_tensor(
                    valid[:], valid[:], selTs[:], op=Alu.mult)

                # ---- own-expert compacted token lists (esel matmul) ----
                for ch in range(T // 512):
                    csl = slice(ch * 512, (ch + 1) * 512)
                    pr4 = psum_t.tile([EL, 512], f32, tag="pt")
                    nc.tensor.matmul(
                        pr4[:], esel_sb[:], rank[:, csl],
                        start=True, stop=True)
                    ps4 = psum_t.tile([EL, 512], f32, tag="pt")
                    nc.tensor.matmul(
                        ps4[:], esel_sb[:], selTs[:, csl],
                        start=True, stop=True)
                    vo = work.tile([EL, 512], f32, tag="vo")
                    nc.vector.tensor_scalar(
                        vo[:], pr4[:], float(C), None, op0=Alu.is_le)
                    nc.vector.tensor_tensor(vo[:], vo[:], ps4[:], op=Alu.mult)
                    io_i = work.tile([EL, 512], dt.int32, tag="ioi")
                    nc.gpsimd.iota(io_i[:], pattern=[[1, 512]],
                                   base=1 + ch * 512, channel_multiplier=0)
                    atc = work.tile([EL, 512], f32, tag="atc")
                    nc.vector.tensor_copy(atc[:], io_i[:])
                    nc.vector.tensor_tensor(atc[:], atc[:], vo[:], op=Alu.mult)
                    nc.vector.tensor_scalar(atc[:], atc[:], 1.0, None,
                                            op0=Alu.subtract)
                    nc.sync.dma_start(at_dram[:, csl], atc[:])
                    # Aw = valid ? w : -1   (w > 0 strictly)
                    pw4 = psum_t.tile([EL, 512], f32, tag="pt")
                    nc.tensor.matmul(
                        pw4[:], esel_sb[:], selTw[:, csl],
                        start=True, stop=True)
                    awc = work.tile([EL, 512], f32, tag="awc")
                    nc.vector.tensor_scalar(
                        awc[:], pw4[:], 1.0, None, op0=Alu.add)
                    nc.vector.tensor_tensor(awc[:], awc[:], vo[:], op=Alu.mult)
                    nc.vector.tensor_scalar(awc[:], awc[:], 1.0, None,
                                            op0=Alu.subtract)
                    nc.sync.dma_start(aw_dram[:, csl], awc[:])

                sgin = route.tile([16, EL, T // 16], f32)
                sginw = route.tile([16, EL, T // 16], f32)
                for e in range(EL):
                    nc.sync.dma_start(
                        sgin[:, e, :],
                        at_dram[e].rearrange("(c b) -> b c", b=16))
                    nc.sync.dma_start(
                        sginw[:, e, :],
                        aw_dram[e].rearrange("(c b) -> b c", b=16))

                idx16s = route.tile([16, EL * IWT], dt.int16)
                nc.gpsimd.memset(idx16s[:], -1)
                sgtoks, sgws = [], []
                for e in range(EL):
                    sgtok = work.tile([16, IW], f32, tag=f"sgtok{e}")
                    nft = work.tile([1, 1], dt.uint32, tag=f"nft{e}")
                    nc.gpsimd.sparse_gather(
                        sgtok[:], sgin[:, e, :], num_found=nft[:])
                    sgw = work.tile([16, IW], f32, tag=f"sgw{e}")
                    nfw = work.tile([1, 1], dt.uint32, tag=f"nfw{e}")
                    nc.gpsimd.sparse_gather(
                        sgw[:], sginw[:, e, :], num_found=nfw[:])
                    nf_f = work.tile([1, 1], f32, tag=f"nf_f{e}")
                    nc.vector.tensor_copy(nf_f[:], nft[:])
                    nc.sync.dma_start(nf_dram[0:1, e:e + 1], nf_f[:])
                    sgtoks.append(sgtok)
                    sgws.append(sgw)

                for e in range(EL):
                    nf16 = work.tile([16, 1], f32, tag=f"nf16{e}")
                    nc.sync.dma_start(
                        nf16[:], nf_dram[0:1, e:e + 1].broadcast_to([16, 1]))
                    vm = work.tile([16, IW], f32, tag=f"vm{e}")
                    nc.vector.tensor_scalar(
                        vm[:], iota_sl[:], nf16[:, 0:1], None, op0=Alu.is_lt)
                    tokm = work.tile([16, IW], f32, tag=f"tokm{e}")
                    nc.vector.tensor_tensor(
                        tokm[:], sgtoks[e][:], vm[:], op=Alu.mult)
                    nc.vector.tensor_scalar(
                        tokm[:], tokm[:], float(T - 1), None, op0=Alu.min)
                    nc.vector.tensor_scalar(
                        tokm[:], tokm[:], 0.0, None, op0=Alu.max)
                    nc.vector.tensor_copy(
                        idx16s[:, IWT * e:IWT * e + IW], tokm[:])
                    # masked per-slot weights -> wrapped-to-linear DRAM
                    wsl = work.tile([16, IW], f32, tag=f"wsl{e}")
                    nc.vector.tensor_tensor(
                        wsl[:], sgws[e][:], vm[:], op=Alu.mult)
                    nc.sync.dma_start(
                        wc_dram[e].rearrange("(c b) -> b c", b=16)
                        [:, 0:IW], wsl[:])

                nc.sync.dma_start(idx_dram[:], idx16s[:])
                for r in range(8):
                    nc.sync.dma_start(
                        idx16[16 * r:16 * (r + 1), :], idx_dram[:])

                # per-slot weights as [128, 2] columns per expert
                for e in range(EL):
                    nc.sync.dma_start(
                        w_col[:, 2 * e:2 * e + 2],
                        wc_dram[e].rearrange("(c p) -> p c", p=128))

            # ---- P6: routed expert GEMMs ----
            with tc.tile_pool(name="dpXPT", bufs=2) as dpXPT, \
                    tc.tile_pool(name="dpHT", bufs=1) as dpHT, \
                    tc.tile_pool(name="dpWD", bufs=2) as dpWD:
                for e in range(EL):
                    # fused gather+transpose: [128 d, DC, 256 slots]; only
                    # cols 0:CP are written/used (idx cols CP..255 are -1)
                    XPT = dpXPT.tile([128, DC, 2 * 128], wdt, tag="XPT")
                    nc.gpsimd.dma_gather(
                        XPT[:], x_bf16[:], idx16[:, IWT * e:IWT * (e + 1)],
                        2 * 128, CP, D, transpose=True, queue_num=0)

                    HT = dpHT.tile([128, IC, CP], wdt, tag="HT")
                    for ic in range(IC):
                        wgu_t = wstream.tile([128, DC, 256], wdt, tag="wst")
                        nc.sync.dma_start(
                            wgu_t[:],
                            wgu[e, ic].rearrange("p (c j) -> p c j", j=256))
                        pg = psum_g.tile([128, CP], f32, tag="pg")
                        pu = psum_u.tile([128, CP], f32, tag="pu")
                        for dc in range(DC):
                            nc.tensor.matmul(
                                pg[:], wgu_t[:, dc, 0:128],
                                XPT[:, dc, 0:CP],
                                start=(dc == 0), stop=(dc == DC - 1))
                        for dc in range(DC):
                            nc.tensor.matmul(
                                pu[:], wgu_t[:, dc, 128:256],
                                XPT[:, dc, 0:CP],
                                start=(dc == 0), stop=(dc == DC - 1))
                        sig = work.tile([128, CP], f32, tag="esig")
                        nc.scalar.activation(sig[:], pg[:], Act.Sigmoid)
                        sil = work.tile([128, CP], wdt, tag="esil")
                        nc.vector.tensor_tensor(
                            sil[:], sig[:], pg[:], op=Alu.mult)
                        nc.vector.tensor_tensor(
                            HT[:, ic, :], sil[:], pu[:], op=Alu.mult)

                    for dc4 in range(D // 512):
                        wd_t = dpWD.tile([128, IC, 512], wdt, tag="wst3")
                        nc.sync.dma_start(
                            wd_t[:],
                            wd[e].rearrange("(c p) d -> p c d", p=128)
                            [:, :, dc4 * 512:(dc4 + 1) * 512])
                        py = psum_y.tile([128, 512], f32, tag="py")
                        for ic in range(IC):
                            nc.tensor.matmul(
                                py[:], HT[:, ic, 0:128], wd_t[:, ic, :],
                                start=(ic == 0), stop=(ic == IC - 1))
                        yw = work.tile([128, 512], f32, tag="yw")
                        nc.vector.tensor_scalar(
                            yw[:], py[:], w_col[:, 2 * e:2 * e + 1], None,
                            op0=Alu.mult)
                        nc.gpsimd.dma_scatter_add(
                            pout[:, dc4 * 512:(dc4 + 1) * 512],
                            yw[:].rearrange("p (a f) -> p a f", a=1),
                            idx16[:, IWT * e:IWT * e + 8],
                            128, 128, 512, elem_step=D, queue_num=0)
                        pyb = psum_y.tile([32, 512], f32, tag="py")
                        for ic in range(IC):
                            nc.tensor.matmul(
                                pyb[:], HT[:, ic, 128:CP], wd_t[:, ic, :],
                                start=(ic == 0), stop=(ic == IC - 1))
                        yw2 = work.tile([128, 512], f32, tag="yw2")
                        nc.gpsimd.memset(yw2[:], 0.0)
                        nc.vector.tensor_scalar(
                            yw2[0:32, :], pyb[:],
                            w_col[0:32, 2 * e + 1:2 * e + 2], None,
                            op0=Alu.mult)
                        nc.gpsimd.dma_scatter_add(
                            pout[:, dc4 * 512:(dc4 + 1) * 512],
                            yw2[:].rearrange("p (a f) -> p a f", a=1),
                            idx16[:, IWT * e + 8:IWT * (e + 1)],
                            128, 32, 512, elem_step=D, queue_num=0)

    nc.finalize()
    return nc


_NC_CACHE = {}


def get_nc(gemm_mode=None):
    gemm_mode = gemm_mode or GEMM_MODE
    if gemm_mode not in _NC_CACHE:
        _NC_CACHE[gemm_mode] = _build(gemm_mode)
    return _NC_CACHE[gemm_mode]


def make_in_maps(inputs, gemm_mode=None):
    """Shard full inputs into the 8 per-core input maps."""
    import ml_dtypes

    gemm_mode = gemm_mode or GEMM_MODE
    wnp = ml_dtypes.bfloat16 if gemm_mode == "bf16" else np.float32

    x = np.asarray(inputs["hidden_states"], np.float32).reshape(T, D)
    router_w = np.asarray(inputs["router_w"], np.float32)
    e_bias = np.asarray(inputs["e_bias"], np.float32).reshape(1, E)
    W_gate = np.asarray(inputs["W_gate"], np.float32)
    W_up = np.asarray(inputs["W_up"], np.float32)
    W_down = np.asarray(inputs["W_down"], np.float32)
    shared_gate = np.asarray(inputs["shared_gate"], np.float32)
    shared_up = np.asarray(inputs["shared_up"], np.float32)
    shared_down = np.asarray(inputs["shared_down"], np.float32)

    x_w = np.ascontiguousarray(x)  # fp32: router source
    x_w16 = np.ascontiguousarray(x).astype(wnp)  # gather source (transpose)
    rwT_np = np.ascontiguousarray(router_w.T.astype(np.float32))

    # routed gate/up packed: [E, IC, 128, (dc, g|u 128+128)]
    Wg5 = W_gate.reshape(E, DC, 128, IC, 128).transpose(0, 3, 2, 1, 4)
    Wu5 = W_up.reshape(E, DC, 128, IC, 128).transpose(0, 3, 2, 1, 4)
    wgu_np = np.concatenate([Wg5, Wu5], axis=4)  # [E, IC, 128, DC, 256]
    wgu_np = np.ascontiguousarray(
        wgu_np.reshape(E, IC, 128, 2 * DC * 128)).astype(wnp)

    # shared gate/up packed: [MC, 128, (dc, g|u)]
    Sg4 = shared_gate.reshape(MC, 128, DC, 128).transpose(0, 3, 2, 1)
    Su4 = shared_up.reshape(MC, 128, DC, 128).transpose(0, 3, 2, 1)
    sgu_np = np.concatenate([Sg4, Su4], axis=3)  # [MC, 128, DC, 256]
    sgu_np = np.ascontiguousarray(
        sgu_np.reshape(MC, 128, 2 * DC * 128)).astype(wnp)

    sdT_np = np.ascontiguousarray(shared_down.T).astype(wnp)

    in_maps = []
    for c in range(NCORES):
        esel_np = np.zeros((E, EL), np.float32)
        for j in range(EL):
            esel_np[EL * c + j, j] = 1.0
        ids = (np.arange(TL, dtype=np.int16) + c * TL)
        tidx_np = np.zeros((128, TL // 16), np.int16)
        for i in range(TL):
            tidx_np[i % 16, i // 16] = ids[i]
        tidx_np = np.tile(tidx_np[:16], (8, 1))
        in_maps.append({
            "x_full": x_w,
            "x_bf16": x_w16,
            "rwT": rwT_np,
            "ebias": e_bias,
            "esel": esel_np,
            "tidx": tidx_np,
            "wgu": np.ascontiguousarray(wgu_np[EL * c:EL * (c + 1)]),
            "wd": np.ascontiguousarray(
                W_down[EL * c:EL * (c + 1)]).astype(wnp),
            "sgu": sgu_np,
            "sdT": sdT_np,
        })
    return in_maps


def kernel(**inputs):
    from concourse.bass_utils import run_bass_kernel_spmd

    nc = get_nc()
    in_maps = make_in_maps(inputs)
    trace = bool(int(os.environ.get("BASS_MOE_TRACE", "0")))
    res = run_bass_kernel_spmd(
        nc, in_maps, core_ids=list(range(NCORES)), trace=trace)
    if trace and res.exec_time_ns is not None:
        print(f"HW exec time: {res.exec_time_ns} ns")
        kernel.last_exec_time_ns = res.exec_time_ns
    full = np.zeros((T, D), np.float64)
    for c in range(NCORES):
        full += res.results[c]["pout"].astype(np.float64)
    for c in range(NCORES):
        full[TL * c:TL * (c + 1)] += res.results[c]["out"].astype(np.float64)
    return full.reshape(B, S, D).astype(np.float32)


kernel.last_exec_time_ns = None
